# revision 27
# baseline (speedup 1.0000x reference)
"""Trainium2 Bass kernel for nn_Attention_9242769622327.

Math: the reference computes
    qkv = x @ W1.T ; q,k,v = split(qkv)
    score = softmax(k^T v / 4, axis=-1)            # rows sum to 1
    attn  = softmax(einsum('bhnk,bhkc->bhnk', q/4, score), axis=-1)
          = softmax(q/4)                           # k/v are mathematically dead
    out   = attn @ W2.T
so only the q-projection (first E rows of W1), a per-head (64-wide) softmax,
and the output projection are needed.

Distribution: pure data-parallel over the 32768 = B*S rows; each of the 8
cores handles 4096 rows. No collectives.

Precision strategy (fp8 DoubleRow = 2 fp8 K-values per PE cell per cycle,
i.e. K=256 per matmul instruction -> half the instruction count):
  mm1 (q-projection) in fp8e4 DoubleRow: x ~ N(0,1) and 32*W1 ~ N(0,1)
      quantize to e4m3 with ~1.8% rel err each -> q abs err ~2.5e-2, and
      exp(q/4) divides it by 4 -> ~0.6% on attn. 32 instrs vs 64 fp16.
  mm2 (output projection) in fp8e4 DoubleRow via CENTERING: softmax over 64
      logits with sigma=0.25 gives at = 64*attn = 1 + delta, |delta|~0.25.
      out = (1/64)*(rowsum(W2T)[j] + delta @ W2T): the constant term is exact
      (per-partition bias on the output copy); only delta rides through fp8,
      so quantization error is ~4x smaller: ~0.45% per operand. 32 instrs.
  head-sum + rcp broadcast stay fp16 (8+8 instrs).

On-chip layout fully transposed (features on partitions, rows on free dim):
    qT[n,m]  = sum_k W1qT[k,n]*xT[k,m]     (PE, fp8 DoubleRow, 32 MM)
    u        = exp(qT/128)  [qT is 32x]    (ACT, PSUM->SBUF fp16)
    u8       = e4m3(u)                     (DVE copy, for the head-sum)
    s[g,m]/64= sum_{n in head g} u8[n,m]   (PE fp8 DoubleRow w/ 1/64 sel, 4 MM)
    rcp      = 64/s                        (DVE reciprocal + fp16 copy)
    rb[n,m]  = rcp[head(n),m]              (DMA broadcast via DRAM bounce:
                                            2 small writes + 2 0-stride reads)
    at       = u * rb                      (DVE, fp16)
    d8       = at - 1                      (DVE tensor_scalar, e4m3 out)
    oT[j,m]  = sum_n 32W2T[n,j]*d8[n,m]    (PE fp8 DoubleRow, 32 MM)
    outT     = oT/2048 + rowsumW2[j]/64    (ACT Identity w/ bias AP, fp16)

Per-stripe PE: 68 matmul instrs vs baseline's 144, in a depth-2 software
pipeline [mm2(ms-2) | mm1(ms) | sum(ms)] so every cross-engine dependency
(exp->sum, DVE delta->mm2, rb DMA round trip, PSUM out-copies) lands with
multi-us slack.  Measured ~158us on 8 cores (~125us pure-matmul floor),
rel err 1.34e-2 (gate 2e-2).
"""

import sys

sys.path.insert(0, "/opt/trn_rl_repo")

import numpy as np
import ml_dtypes

import concourse.bass as bass
import concourse.bacc as bacc
import concourse.tile as tile
from concourse import mybir
from concourse.bass_utils import run_bass_kernel_spmd

FP16 = mybir.dt.float16
FP8 = mybir.dt.float8e4
F32 = mybir.dt.float32
AF = mybir.ActivationFunctionType
DR = mybir.MatmulPerfMode.DoubleRow

N_CORES = 8
B, S, E = 4, 8192, 1024
HEADS, HEAD_DIM = 16, 64
M_TOTAL = B * S                # 32768
M_CORE = M_TOTAL // N_CORES    # 4096 rows per core
MS = 512                       # m-stripe width (moving free dim / PSUM bank)
N_STRIPES = M_CORE // MS       # 8
KC2 = E // 256                 # 4 DoubleRow contraction chunks (K=256 each)
NC_ = E // 128                 # 8 feature chunks

_E4 = ml_dtypes.float8_e4m3
_F16 = np.float16


def build_nc() -> bass.Bass:
    nc = bacc.Bacc("TRN2", debug=False)

    xt = nc.dram_tensor("xt", [E, M_CORE], FP8, kind="ExternalInput")
    w1 = nc.dram_tensor("w1", [E, E], FP8, kind="ExternalInput")
    w2 = nc.dram_tensor("w2", [E, E], FP8, kind="ExternalInput")
    sel = nc.dram_tensor("sel", [128, KC2 * 2 * HEADS], FP8, kind="ExternalInput")
    bias = nc.dram_tensor("bias", [128, NC_], F32, kind="ExternalInput")
    outT = nc.dram_tensor("outT", [E, M_CORE], FP16, kind="ExternalOutput")
    # per-stripe DRAM scratch for the rcp broadcast bounce (even/odd heads)
    scr_e = [
        nc.dram_tensor(f"scr_e{ms}", [NC_, MS], FP16, kind="Internal")
        for ms in range(N_STRIPES)
    ]
    scr_o = [
        nc.dram_tensor(f"scr_o{ms}", [NC_, MS], FP16, kind="Internal")
        for ms in range(N_STRIPES)
    ]

    # row k of xt/w1 maps to (c, i, p): k = c*256 + i*128 + p  (DoubleRow pair
    # slot i); same for w2 rows n = t*256 + i*128 + p.
    xt_v = xt[:, :].rearrange("(c i p) m -> p c i m", p=128, i=2)
    w1_v = w1[:, :].rearrange("(c i p) n -> p c i n", p=128, i=2)
    w2_v = w2[:, :].rearrange("(t i p) j -> p t i j", p=128, i=2)

    with tile.TileContext(nc) as tc:
        with (
            tc.tile_pool(name="weights", bufs=1) as wpool,
            tc.tile_pool(name="xt", bufs=N_STRIPES) as xpool,
            tc.tile_pool(name="u", bufs=20) as upool,
            tc.tile_pool(name="at", bufs=20) as apool,
            tc.tile_pool(name="d8", bufs=12) as dpool,
            tc.tile_pool(name="u8", bufs=8) as u8pool,
            tc.tile_pool(name="small", bufs=3) as spool,
            tc.tile_pool(name="rb", bufs=2) as rbpool,
            tc.tile_pool(name="ostage", bufs=12) as opool,
            tc.tile_pool(name="ps_q", bufs=4, space="PSUM") as psq,
            tc.tile_pool(name="ps_s", bufs=1, space="PSUM") as pss,
            tc.tile_pool(name="ps_o", bufs=3, space="PSUM") as pso,
        ):
            # Warm the PE's HAM clock gate with throwaway matmuls on memset
            # scratch while the first weight/x DMAs are in flight.
            warm_sb = wpool.tile([128, MS], FP16, name="warm_sb")
            nc.gpsimd.memset(warm_sb[:], 0.0)
            warm_ps = psq.tile([128, MS], F32, tag="q", name="warm_ps")
            for _ in range(8):
                nc.tensor.matmul(
                    warm_ps[:], warm_sb[:, 0:128], warm_sb[:], start=True, stop=True
                )

            # Load order: w1 + sel chunks (needed by stripe 0's mm1/sum),
            # stripe-0 x chunks, then w2/selt/bias (needed ~15us in).
            w1_c = []
            xt0 = []
            for c in range(KC2):
                t = wpool.tile([128, 2, E], FP8, tag=f"w1_{c}", name=f"w1c{c}")
                nc.sync.dma_start(t[:], w1_v[:, c, :, :])
                w1_c.append(t)
                tx = xpool.tile([128, 2, MS], FP8, tag=f"xt_{c}", name=f"xt0_{c}")
                nc.sync.dma_start(tx[:], xt_v[:, c, :, 0:MS])
                xt0.append(tx)
            sel_t = wpool.tile([128, KC2, 2, HEADS], FP8, name="sel_t")
            nc.sync.dma_start(
                sel_t[:], sel[:, :].rearrange("p (t i g) -> p t i g", i=2, g=HEADS)
            )

            w2_c = []
            for tt in range(KC2):
                t = wpool.tile([128, 2, E], FP8, tag=f"w2_{tt}", name=f"w2c{tt}")
                nc.sync.dma_start(t[:], w2_v[:, tt, :, :])
                w2_c.append(t)
            bias_t = wpool.tile([128, NC_], F32, name="bias_t")
            nc.sync.dma_start(bias_t[:], bias[:, :])

            # Software pipeline over stripes: while stripe ms runs mm1 + exp +
            # head-sum, stripe ms-1 runs normalization (rb + DVE mul + delta
            # cast) and the output projection, so the PE never waits on the
            # softmax chain.
            prev_u = None
            prev_rb = None
            prev_ms = -1

            def emit_norm(pu, prb):
                """DVE mul (at=u*rb, rb from the DMA broadcast) + delta = at-1
                cast to e4m3 pair tiles for the DoubleRow mm2."""
                dts = []
                for tt in range(KC2):
                    dt_t = dpool.tile([128, 2, MS], FP8, tag=f"d8_{tt}", name=f"d8_{tt}")
                    dts.append(dt_t)
                for ci in range(NC_):
                    at_t = apool.tile([128, MS], FP16, tag="at", name="at_t")
                    nc.vector.tensor_mul(at_t[:], pu[ci][:], prb[:, ci, :])
                    nc.vector.tensor_scalar_sub(
                        dts[ci // 2][:, ci % 2, :], at_t[:], 1.0
                    )
                return dts

            def emit_tail(dts, ms):
                """mm2 (DoubleRow over delta pairs) + biased out-copy + store."""
                for j in range(NC_):
                    o_ps = pso.tile([128, MS], F32, tag="o", name="o_ps")
                    for tt in range(KC2):
                        nc.tensor.matmul(
                            o_ps[:],
                            w2_c[tt][:, :, j * 128:(j + 1) * 128],
                            dts[tt][:],
                            start=(tt == 0),
                            stop=(tt == KC2 - 1),
                            perf_mode=DR,
                        )
                    o_t = opool.tile([128, MS], FP16, tag="ost", name="o_t")
                    # out = o_ps/2048 + rowsum(W2T)[j]/64 (ACT; drains during
                    # the following mm1 block, before that stripe's exps).
                    # For the last stripe there is no following mm1 to hide
                    # under, so alternate ACT/DVE and both DMA queues to
                    # shorten the drain.
                    last = ms == N_STRIPES - 1
                    if last and j % 2 == 1:
                        nc.vector.tensor_scalar(
                            o_t[:], o_ps[:], 1.0 / 2048.0, bias_t[:, j:j + 1],
                            mybir.AluOpType.mult, mybir.AluOpType.add,
                        )
                    else:
                        nc.scalar.activation(
                            o_t[:], o_ps[:], AF.Identity,
                            bias=bias_t[:, j:j + 1], scale=1.0 / 2048.0,
                        )
                    # gpsimd queue keeps Sync free mid-kernel, but its ring
                    # quiesces slowly in the final DRAIN -- so the last stripe
                    # goes on Sync, letting the gpsimd ring empty a stripe
                    # before the kernel ends.
                    q = nc.sync if last else nc.gpsimd
                    q.dma_start(
                        outT[j * 128:(j + 1) * 128, ms * MS:(ms + 1) * MS], o_t[:]
                    )

            # Depth-2 pipeline, mm2 leads each iteration:
            #   PE:  [mm2(ms-2) 32][mm1(ms) 32][sum(ms) 8]
            #   ACT: [out-copies(ms-2) x8][exp(ms) x8]
            #   DVE: [mul+sub(ms-1) x16][recip+cast(ms)]
            # so every cross-engine dependency lands with multi-us slack:
            # mm2's delta tiles were finished one full iteration earlier, the
            # out-copies drain while mm1 runs, and the rb broadcast DMA has
            # most of an iteration to do its DRAM round trip.
            prev_d = None
            prev_d_ms = -1
            for ms in range(N_STRIPES):
                if ms == 0:
                    xt_k = xt0
                else:
                    xt_k = []
                    for c in range(KC2):
                        t = xpool.tile(
                            [128, 2, MS], FP8, tag=f"xt_{c}", name=f"xt{ms}_{c}"
                        )
                        nc.sync.dma_start(
                            t[:], xt_v[:, c, :, ms * MS:(ms + 1) * MS]
                        )
                        xt_k.append(t)

                # ---- stripe ms-2 output projection (delta ready long ago) ----
                if prev_d is not None:
                    emit_tail(prev_d, prev_d_ms)
                # ---- stripe ms-1 normalization (DVE only) ----
                if prev_rb is not None:
                    prev_d = emit_norm(prev_u, prev_rb)
                    prev_d_ms = prev_ms

                # ---- mm1: q-projection, 32 contiguous DoubleRow MMs ----
                u_tiles = []
                u8_tiles = [
                    u8pool.tile([128, 2, MS], FP8, tag=f"u8_{t}", name=f"u8_{t}")
                    for t in range(KC2)
                ]
                # head-sum instrs are interleaved into the mm1 block (the
                # s_ps accumulation group runs on its own PSUM bank, so it can
                # interleave with the q-chains): sum instr t follows chunk
                # 2t+1, letting recip -> rb-DMA launch ~1.5us earlier.
                s_ps = pss.tile([HEADS, MS], F32, tag="s", name="s_ps")
                for ci in range(NC_):
                    q_ps = psq.tile([128, MS], F32, tag="q", name="q_ps")
                    for c in range(KC2):
                        nc.tensor.matmul(
                            q_ps[:],
                            w1_c[c][:, :, ci * 128:(ci + 1) * 128],
                            xt_k[c][:],
                            start=(c == 0),
                            stop=(c == KC2 - 1),
                            perf_mode=DR,
                        )
                    u_t = upool.tile([128, MS], FP16, tag="u", name="u_t")
                    nc.scalar.activation(u_t[:], q_ps[:], AF.Exp, scale=1.0 / 128.0)
                    u_tiles.append(u_t)
                    # e4m3 copy for the DoubleRow head-sum (sum err ~2.5%/8)
                    nc.vector.tensor_copy(
                        u8_tiles[ci // 2][:, ci % 2, :], u_t[:]
                    )
                    if ci % 2 == 1:
                        t = ci // 2
                        nc.tensor.matmul(
                            s_ps[:],
                            sel_t[:, t, :, :],
                            u8_tiles[t][:],
                            start=(t == 0),
                            stop=(t == KC2 - 1),
                            perf_mode=DR,
                            skip_group_check=True,
                        )
                rcp32 = spool.tile([HEADS, MS], F32, tag="rcp32", name="rcp32")
                nc.vector.reciprocal_approx_fast(rcp32[:], s_ps[:])
                rcp_t = spool.tile([HEADS, MS], FP16, tag="rcp", name="rcp_t")
                nc.vector.tensor_copy(rcp_t[:], rcp32[:])
                # broadcast rcp rows to all 8 chunks via a DRAM bounce: two
                # small writes (even/odd head rows), then two 0-stride reads
                # fill rb[p, ci, m] = rcp[2ci + (p>=64), m].  All on the same
                # DMA queue, so write->read ordering holds; the result isn't
                # consumed until next stripe's norm, hiding the latency.
                rb_t = rbpool.tile([128, NC_, MS], FP16, tag="rb", name="rb_t")
                nc.sync.dma_start(scr_e[ms][:, :], rcp_t[0:HEADS:2, :])
                nc.sync.dma_start(scr_o[ms][:, :], rcp_t[1:HEADS:2, :])
                nc.sync.dma_start(
                    rb_t[0:64, :, :],
                    scr_e[ms][:, :].unsqueeze(0).broadcast_to([64, NC_, MS]),
                )
                nc.sync.dma_start(
                    rb_t[64:128, :, :],
                    scr_o[ms][:, :].unsqueeze(0).broadcast_to([64, NC_, MS]),
                )

                prev_u, prev_rb, prev_ms = u_tiles, rb_t, ms

            # epilogue: drain the depth-2 pipeline.  norm(last) goes first
            # so its DVE mul/sub chain (waiting on the rb broadcast DMA) runs
            # underneath mm2(last-1) on the PE.
            d_tiles = emit_norm(prev_u, prev_rb)
            emit_tail(prev_d, prev_d_ms)
            emit_tail(d_tiles, prev_ms)
    nc.compile()
    return nc


_NC_CACHE = None
LAST_RESULT = None


def _ensure_ntff_hook():
    """bass_utils' axon trace path needs antenv.axon_hooks, which this
    container's antenv lacks. Provide it + register the ctypes NTFF hook."""
    import types

    try:
        from antenv.axon_hooks import get_axon_ntff_profile_hook  # noqa: F401
        return True
    except ImportError:
        pass
    try:
        import antenv
        from trn_agent_boot.trn_boot import _ntff_profile_via_ctypes

        m = types.ModuleType("antenv.axon_hooks")
        state = {"hook": None}
        m.set_axon_ntff_profile_hook = lambda h: state.__setitem__("hook", h)
        m.get_axon_ntff_profile_hook = lambda: state["hook"]
        sys.modules["antenv.axon_hooks"] = m
        antenv.axon_hooks = m
        m.set_axon_ntff_profile_hook(
            _ntff_profile_via_ctypes("/opt/axon/libaxon_pjrt.so")
        )
        return True
    except Exception as e:  # pragma: no cover
        print(f"ntff hook injection failed: {e}")
        return False


def _selectors():
    # DoubleRow pair selector: u8[t][p, i, m] holds feature n = (2t+i)*128+p,
    # whose head is 4t + 2i + (p>=64).  Entries are 1/64 (exact in e4m3) so
    # the sum comes out pre-scaled (s/64) and reciprocal gives 64/s directly.
    sel = np.zeros((128, KC2, 2, HEADS), np.float32)
    for t in range(KC2):
        for i in range(2):
            sel[:64, t, i, 4 * t + 2 * i] = 1.0 / 64.0
            sel[64:, t, i, 4 * t + 2 * i + 1] = 1.0 / 64.0
    return np.ascontiguousarray(sel.reshape(128, KC2 * 2 * HEADS)).astype(_E4)


def kernel(x, W1, W2, heads, trace=False):
    global _NC_CACHE, LAST_RESULT
    x = np.asarray(x, dtype=np.float32)
    W1 = np.asarray(W1, dtype=np.float32)
    W2 = np.asarray(W2, dtype=np.float32)

    X = x.reshape(M_TOTAL, E)
    Xq = X.astype(_E4)
    XqT = Xq.T  # [E, M_TOTAL] view
    w1q = np.ascontiguousarray(32.0 * W1[:E, :].T).astype(_E4)   # [k, n]
    w2q = np.ascontiguousarray(32.0 * W2.T).astype(_E4)          # [n, j]
    # bias[p, j] = rowsum(W2T)[j*128+p] / 64
    bias = np.ascontiguousarray(
        (W2.sum(axis=1) / 64.0).reshape(NC_, 128).T
    ).astype(np.float32)
    sel = _selectors()

    in_maps = []
    for c in range(N_CORES):
        xt_c = np.ascontiguousarray(XqT[:, c * M_CORE:(c + 1) * M_CORE])
        in_maps.append(
            {"xt": xt_c, "w1": w1q, "w2": w2q, "sel": sel, "bias": bias}
        )

    if _NC_CACHE is None:
        _NC_CACHE = build_nc()

    if trace:
        trace = _ensure_ntff_hook()

    res = run_bass_kernel_spmd(_NC_CACHE, in_maps, list(range(N_CORES)), trace=trace)
    LAST_RESULT = res

    OT = np.concatenate(
        [np.asarray(res.results[c]["outT"]).astype(np.float32) for c in range(N_CORES)],
        axis=1,
    )
    return np.ascontiguousarray(OT.T).reshape(B, S, E)
